# revision 1
# baseline (speedup 1.0000x reference)
"""Trainium2 Bass kernel for the minGRU-style log-space scan.

Reference computation (B=16, T=4096, H=1024):
    a_star = pad(cumsum(log_coeffs, t))                      # (B, T+1, H)
    log_h  = a_star + cumlogsumexp(log_values - a_star, t)   # (B, T+1, H)
    out    = exp(log_h[:, 1:])                               # (B, T, H)

which is exactly the first-order linear recurrence in linear space:
    h_0 = exp(log_values[:, 0])
    h_t = exp(log_coeffs[:, t-1]) * h_{t-1} + exp(log_values[:, t])
    out[:, t-1] = h_t
(coefficients lie in (exp(-1), 1) and values are lognormal, so h stays
bounded ~O(100); fp32 linear-space evaluation matches the log-space
reference to ~1e-4 scale-relative error.)

Device mapping: each of the B*H = 16384 (batch, hidden) pairs is an
independent length-T recurrence. We transpose host-side to (B*H, T)
row-major, shard 2048 rows to each of the 8 cores, and on each core run
the recurrence with rows on SBUF partitions and time on the free
dimension using the VectorE `tensor_tensor_scan` instruction
(state = c * state + v along the free dim, fp32 state, per-partition
initial). ScalarE does the two exps; SyncE issues input DMA, GpSimdE
issues output DMA.  All DMA is contiguous; the kernel is DMA-bound
(~100 MB/core @ ~360 GB/s).

Raw-bass pipeline (no Tile): NBUF-deep ring buffers per stream with
explicit semaphores.  Per chunk i:
    SP:   [wait slot free]  load lc_i -> lcbuf,  load lv_i -> lvbuf
    ACT:  [wait lc_i]  c_i = exp(lc_i)   [wait lv_i]  v_i = exp(lv_i)
    DVE:  [wait c_i,v_i; wait hbuf free] h_i = scan(c_i, v_i, init)
    POOL: [wait h_i] store h_i -> out
`init` is v_i[:,0:1] for a group's first chunk, else h_{i-1}[:,-1:]
(consecutive scans of one row-group run in order on the DVE).
"""

import contextlib

import numpy as np

import concourse.bass as bass
import concourse.mybir as mybir
from concourse.bass_utils import run_bass_kernel_spmd

B, T, H = 16, 4096, 1024
N_CORES = 8
ROWS = B * H // N_CORES  # 2048 rows (sequences) per core
F32 = mybir.dt.float32


def build_nc_inplace(rows: int = ROWS, t: int = T, tc: int = 2048,
                     repeat: int = 1, nbuf: int = 6) -> bass.Bass:
    """Variant of build_nc with exp computed in place (c overwrites the
    lc tile, v overwrites the lv tile): 3 SBUF streams instead of 5, so
    the rings can be deeper / chunks bigger."""
    assert rows % 128 == 0 and t % tc == 0 and nbuf >= 2
    nc = bass.Bass()
    lc = nc.declare_dram_parameter("lc", [rows, t], F32, isOutput=False)
    lv = nc.declare_dram_parameter("lv", [rows, t + 1], F32, isOutput=False)
    out = nc.declare_dram_parameter("out", [rows, t], F32, isOutput=True)

    n_groups = rows // 128
    n_chunks = t // tc
    n_iters = repeat * n_groups * n_chunks
    exp = mybir.ActivationFunctionType.Exp
    sched = [(g, k) for _ in range(repeat) for g in range(n_groups)
             for k in range(n_chunks)]

    with contextlib.ExitStack() as ctx:
        def sb(name, width):
            return [ctx.enter_context(
                nc.sbuf_tensor(f"{name}{j}", [128, width], F32))
                for j in range(nbuf)]

        cbuf = sb("cbuf", tc)       # holds log_coeffs, then exp'd in place
        vbuf = sb("vbuf", tc + 1)   # holds log_values, then exp'd in place
        hbuf = sb("hbuf", tc)
        lc_sem = [ctx.enter_context(nc.semaphore(f"lc_sem{j}")) for j in range(nbuf)]
        lv_sem = [ctx.enter_context(nc.semaphore(f"lv_sem{j}")) for j in range(nbuf)]
        out_sem = [ctx.enter_context(nc.semaphore(f"out_sem{j}")) for j in range(nbuf)]
        act_sem = ctx.enter_context(nc.semaphore("act_sem"))
        scan_sem = ctx.enter_context(nc.semaphore("scan_sem"))
        block = ctx.enter_context(nc.Block())

        @block.sync
        def _(sync: bass.BassEngine):
            for i, (g, k) in enumerate(sched):
                rs, c0 = slice(g * 128, (g + 1) * 128), k * tc
                b = i % nbuf
                if i >= nbuf:
                    # cbuf[b]/vbuf[b] last read by scan i-nbuf
                    sync.wait_ge(scan_sem, i - nbuf + 1)
                sync.dma_start(out=cbuf[b][:, :], in_=lc[rs, c0:c0 + tc]).then_inc(lc_sem[b], 16)
                sync.dma_start(out=vbuf[b][:, :], in_=lv[rs, c0:c0 + tc + 1]).then_inc(lv_sem[b], 16)

        @block.scalar
        def _(scalar: bass.BassEngine):
            for i, (g, k) in enumerate(sched):
                b = i % nbuf
                scalar.wait_ge(lc_sem[b], 16 * (i // nbuf + 1))
                nc.scalar.activation(cbuf[b][:, :], cbuf[b][:, :], exp).then_inc(act_sem, 1)
                scalar.wait_ge(lv_sem[b], 16 * (i // nbuf + 1))
                nc.scalar.activation(vbuf[b][:, :], vbuf[b][:, :], exp).then_inc(act_sem, 1)

        @block.vector
        def _(vector: bass.BassEngine):
            for i, (g, k) in enumerate(sched):
                b = i % nbuf
                vector.wait_ge(act_sem, 2 * i + 2)
                if i >= nbuf:
                    # hbuf[b] last read by store i-nbuf
                    vector.wait_ge(out_sem[b], 16 * (i // nbuf))
                if k != 0 and i > 0:
                    # force predecessor-scan completion before this
                    # instruction's per-partition `initial` operand is
                    # prefetched at decode time.
                    vector.wait_ge(scan_sem, i)
                init = vbuf[b][:, 0:1] if k == 0 else hbuf[(i - 1) % nbuf][:, tc - 1:tc]
                nc.vector.tensor_tensor_scan(
                    hbuf[b][:, :], cbuf[b][:, :], vbuf[b][:, 1:tc + 1], init,
                    mybir.AluOpType.mult, mybir.AluOpType.add,
                ).then_inc(scan_sem, 1)

        @block.gpsimd
        def _(gpsimd: bass.BassEngine):
            for i, (g, k) in enumerate(sched):
                rs, c0 = slice(g * 128, (g + 1) * 128), k * tc
                b = i % nbuf
                gpsimd.wait_ge(scan_sem, i + 1)
                gpsimd.dma_start(out=out[rs, c0:c0 + tc], in_=hbuf[b][:, :]).then_inc(out_sem[b], 16)
            for j in range(nbuf):
                rounds = (n_iters - 1 - j) // nbuf + 1 if j < n_iters else 0
                if rounds:
                    gpsimd.wait_ge(out_sem[j], 16 * rounds)

    return nc


def build_nc(rows: int = ROWS, t: int = T, tc: int = 2048,
             repeat: int = 1, nbuf: int = 4) -> bass.Bass:
    """Per-core SPMD program.

    Inputs:  lc (rows, t)     log_coeffs, time-major rows
             lv (rows, t+1)   log_values, time-major rows
    Output:  out (rows, t)    h_1..h_t per row

    `repeat` re-emits the program body (for wall-clock timing); the
    result is idempotent.
    """
    assert rows % 128 == 0 and t % tc == 0 and nbuf >= 2
    nc = bass.Bass()
    lc = nc.declare_dram_parameter("lc", [rows, t], F32, isOutput=False)
    lv = nc.declare_dram_parameter("lv", [rows, t + 1], F32, isOutput=False)
    out = nc.declare_dram_parameter("out", [rows, t], F32, isOutput=True)

    n_groups = rows // 128
    n_chunks = t // tc
    n_iters = repeat * n_groups * n_chunks
    exp = mybir.ActivationFunctionType.Exp

    # iteration schedule: (group, chunk) pairs in order, `repeat` times
    sched = [(g, k) for _ in range(repeat) for g in range(n_groups)
             for k in range(n_chunks)]

    with contextlib.ExitStack() as ctx:
        def sb(name, width):
            return [ctx.enter_context(
                nc.sbuf_tensor(f"{name}{j}", [128, width], F32))
                for j in range(nbuf)]

        lcbuf = sb("lcbuf", tc)
        lvbuf = sb("lvbuf", tc + 1)
        cbuf = sb("cbuf", tc)
        vbuf = sb("vbuf", tc + 1)
        hbuf = sb("hbuf", tc)
        # DMA completions are NOT ordered across queues, so a single
        # counting semaphore per stream is ambiguous ("N increments
        # happened" does not imply DMAs 0..N-1 specifically completed).
        # One semaphore per ring slot -> at most one outstanding DMA per
        # semaphore -> the count is exact.
        lc_sem = [ctx.enter_context(nc.semaphore(f"lc_sem{j}")) for j in range(nbuf)]
        lv_sem = [ctx.enter_context(nc.semaphore(f"lv_sem{j}")) for j in range(nbuf)]
        out_sem = [ctx.enter_context(nc.semaphore(f"out_sem{j}")) for j in range(nbuf)]
        act_sem = ctx.enter_context(nc.semaphore("act_sem"))
        scan_sem = ctx.enter_context(nc.semaphore("scan_sem"))
        block = ctx.enter_context(nc.Block())

        @block.sync
        def _(sync: bass.BassEngine):
            for i, (g, k) in enumerate(sched):
                rs, c0 = slice(g * 128, (g + 1) * 128), k * tc
                b = i % nbuf
                if i >= nbuf:
                    # lcbuf[b]/lvbuf[b] last read by exp pair i-nbuf
                    sync.wait_ge(act_sem, 2 * (i - nbuf) + 1)
                sync.dma_start(out=lcbuf[b][:, :], in_=lc[rs, c0:c0 + tc]).then_inc(lc_sem[b], 16)
                if i >= nbuf:
                    sync.wait_ge(act_sem, 2 * (i - nbuf) + 2)
                sync.dma_start(out=lvbuf[b][:, :], in_=lv[rs, c0:c0 + tc + 1]).then_inc(lv_sem[b], 16)

        @block.scalar
        def _(scalar: bass.BassEngine):
            for i, (g, k) in enumerate(sched):
                b = i % nbuf
                scalar.wait_ge(lc_sem[b], 16 * (i // nbuf + 1))
                if i >= nbuf:
                    # cbuf[b]/vbuf[b] last read by scan i-nbuf
                    scalar.wait_ge(scan_sem, i - nbuf + 1)
                nc.scalar.activation(cbuf[b][:, :], lcbuf[b][:, :], exp).then_inc(act_sem, 1)
                scalar.wait_ge(lv_sem[b], 16 * (i // nbuf + 1))
                nc.scalar.activation(vbuf[b][:, :], lvbuf[b][:, :], exp).then_inc(act_sem, 1)

        @block.vector
        def _(vector: bass.BassEngine):
            for i, (g, k) in enumerate(sched):
                b = i % nbuf
                vector.wait_ge(act_sem, 2 * i + 2)
                if i >= nbuf:
                    # hbuf[b] last read by store i-nbuf
                    vector.wait_ge(out_sem[b], 16 * (i // nbuf))
                if k != 0 and i > 0:
                    # the NX prefetches the scan's per-partition `initial`
                    # operand at decode time; force full completion of the
                    # predecessor scan (its tail writes the very column we
                    # read) before this instruction is decoded.
                    vector.wait_ge(scan_sem, i)
                init = vbuf[b][:, 0:1] if k == 0 else hbuf[(i - 1) % nbuf][:, tc - 1:tc]
                nc.vector.tensor_tensor_scan(
                    hbuf[b][:, :], cbuf[b][:, :], vbuf[b][:, 1:tc + 1], init,
                    mybir.AluOpType.mult, mybir.AluOpType.add,
                ).then_inc(scan_sem, 1)

        @block.gpsimd
        def _(gpsimd: bass.BassEngine):
            for i, (g, k) in enumerate(sched):
                rs, c0 = slice(g * 128, (g + 1) * 128), k * tc
                b = i % nbuf
                gpsimd.wait_ge(scan_sem, i + 1)
                gpsimd.dma_start(out=out[rs, c0:c0 + tc], in_=hbuf[b][:, :]).then_inc(out_sem[b], 16)
            for j in range(nbuf):
                rounds = (n_iters - 1 - j) // nbuf + 1 if j < n_iters else 0
                if rounds:
                    gpsimd.wait_ge(out_sem[j], 16 * rounds)

    return nc


def _shard_inputs(log_coeffs: np.ndarray, log_values: np.ndarray):
    """(B,T,H)/(B,T+1,H) -> per-core row-major (rows, time) shards."""
    lct = np.swapaxes(log_coeffs, 1, 2).reshape(B * H, T)
    lvt = np.swapaxes(log_values, 1, 2).reshape(B * H, T + 1)
    lct = np.ascontiguousarray(lct, dtype=np.float32)
    lvt = np.ascontiguousarray(lvt, dtype=np.float32)
    return [
        {"lc": lct[i * ROWS:(i + 1) * ROWS], "lv": lvt[i * ROWS:(i + 1) * ROWS]}
        for i in range(N_CORES)
    ]


def build_nc_split(rows: int = ROWS, t: int = T, tc: int = 2048,
                   repeat: int = 1, nbuf: int = 8) -> bass.Bass:
    """Like build_nc_inplace, but lv loads issue from the ScalarE HWDGE
    ring (software-pipelined with lookahead nbuf//2) so the two input
    streams ride different DMA rings."""
    assert rows % 128 == 0 and t % tc == 0 and nbuf >= 4
    nc = bass.Bass()
    lc = nc.declare_dram_parameter("lc", [rows, t], F32, isOutput=False)
    lv = nc.declare_dram_parameter("lv", [rows, t + 1], F32, isOutput=False)
    out = nc.declare_dram_parameter("out", [rows, t], F32, isOutput=True)

    n_groups = rows // 128
    n_chunks = t // tc
    n_iters = repeat * n_groups * n_chunks
    look = nbuf // 2
    exp = mybir.ActivationFunctionType.Exp
    sched = [(g, k) for _ in range(repeat) for g in range(n_groups)
             for k in range(n_chunks)]

    with contextlib.ExitStack() as ctx:
        def sb(name, width):
            return [ctx.enter_context(
                nc.sbuf_tensor(f"{name}{j}", [128, width], F32))
                for j in range(nbuf)]

        cbuf = sb("cbuf", tc)
        vbuf = sb("vbuf", tc + 1)
        hbuf = sb("hbuf", tc)
        lc_sem = [ctx.enter_context(nc.semaphore(f"lc_sem{j}")) for j in range(nbuf)]
        lv_sem = [ctx.enter_context(nc.semaphore(f"lv_sem{j}")) for j in range(nbuf)]
        out_sem = [ctx.enter_context(nc.semaphore(f"out_sem{j}")) for j in range(nbuf)]
        act_sem = ctx.enter_context(nc.semaphore("act_sem"))
        scan_sem = ctx.enter_context(nc.semaphore("scan_sem"))
        block = ctx.enter_context(nc.Block())

        def lv_load(eng, j):
            g, k = sched[j]
            rs, c0 = slice(g * 128, (g + 1) * 128), k * tc
            bj = j % nbuf
            need = j - nbuf + 1  # vbuf[bj] last read by scan j-nbuf
            if need > 0:
                eng.wait_ge(scan_sem, need)
            eng.dma_start(out=vbuf[bj][:, :], in_=lv[rs, c0:c0 + tc + 1]).then_inc(lv_sem[bj], 16)

        @block.sync
        def _(sync: bass.BassEngine):
            for i, (g, k) in enumerate(sched):
                rs, c0 = slice(g * 128, (g + 1) * 128), k * tc
                b = i % nbuf
                if i >= nbuf:
                    sync.wait_ge(scan_sem, i - nbuf + 1)
                sync.dma_start(out=cbuf[b][:, :], in_=lc[rs, c0:c0 + tc]).then_inc(lc_sem[b], 16)

        @block.scalar
        def _(scalar: bass.BassEngine):
            for j in range(min(look, n_iters)):
                lv_load(scalar, j)
            for i, (g, k) in enumerate(sched):
                b = i % nbuf
                if i + look < n_iters:
                    lv_load(scalar, i + look)
                scalar.wait_ge(lc_sem[b], 16 * (i // nbuf + 1))
                nc.scalar.activation(cbuf[b][:, :], cbuf[b][:, :], exp).then_inc(act_sem, 1)
                scalar.wait_ge(lv_sem[b], 16 * (i // nbuf + 1))
                nc.scalar.activation(vbuf[b][:, :], vbuf[b][:, :], exp).then_inc(act_sem, 1)

        @block.vector
        def _(vector: bass.BassEngine):
            for i, (g, k) in enumerate(sched):
                b = i % nbuf
                vector.wait_ge(act_sem, 2 * i + 2)
                if i >= nbuf:
                    vector.wait_ge(out_sem[b], 16 * (i // nbuf))
                if k != 0 and i > 0:
                    vector.wait_ge(scan_sem, i)
                init = vbuf[b][:, 0:1] if k == 0 else hbuf[(i - 1) % nbuf][:, tc - 1:tc]
                nc.vector.tensor_tensor_scan(
                    hbuf[b][:, :], cbuf[b][:, :], vbuf[b][:, 1:tc + 1], init,
                    mybir.AluOpType.mult, mybir.AluOpType.add,
                ).then_inc(scan_sem, 1)

        @block.gpsimd
        def _(gpsimd: bass.BassEngine):
            for i, (g, k) in enumerate(sched):
                rs, c0 = slice(g * 128, (g + 1) * 128), k * tc
                b = i % nbuf
                gpsimd.wait_ge(scan_sem, i + 1)
                gpsimd.dma_start(out=out[rs, c0:c0 + tc], in_=hbuf[b][:, :]).then_inc(out_sem[b], 16)
            for j in range(nbuf):
                rounds = (n_iters - 1 - j) // nbuf + 1 if j < n_iters else 0
                if rounds:
                    gpsimd.wait_ge(out_sem[j], 16 * rounds)

    return nc


def default_build(repeat: int = 1) -> bass.Bass:
    """Best measured config: in-place exp, 2048-wide chunks, 8-deep rings
    (~291 us/pass vs ~279 us DMA-only floor, ~268 us HBM roofline)."""
    return build_nc_inplace(tc=2048, nbuf=8, repeat=repeat)


def kernel(log_coeffs: np.ndarray, log_values: np.ndarray) -> np.ndarray:
    in_maps = _shard_inputs(log_coeffs, log_values)
    nc = default_build()
    try:
        results = run_bass_kernel_spmd(nc, in_maps, list(range(N_CORES))).results
    except Exception:
        # the shared device pool occasionally comes up wedged from a prior
        # process (NRT_EXEC_UNIT_UNRECOVERABLE); one retry clears it
        import time as _time
        _time.sleep(15)
        results = run_bass_kernel_spmd(nc, in_maps, list(range(N_CORES))).results
    full = np.concatenate([r["out"] for r in results], axis=0)  # (B*H, T)
    out = np.swapaxes(full.reshape(B, H, T), 1, 2)  # (B, T, H) strided view
    return np.ascontiguousarray(out)



# revision 2
# speedup vs baseline: 2.3398x; 2.3398x over previous
"""Trainium2 Bass kernel for the minGRU-style log-space scan.

Reference computation (B=16, T=4096, H=1024):
    a_star = pad(cumsum(log_coeffs, t))                      # (B, T+1, H)
    log_h  = a_star + cumlogsumexp(log_values - a_star, t)   # (B, T+1, H)
    out    = exp(log_h[:, 1:])                               # (B, T, H)

which is exactly the first-order linear recurrence in linear space:
    h_0 = exp(log_values[:, 0])
    h_t = exp(log_coeffs[:, t-1]) * h_{t-1} + exp(log_values[:, t])
    out[:, t-1] = h_t
(coefficients lie in (exp(-1), 1) and values are lognormal, so h stays
bounded ~O(100); linear-space evaluation matches the log-space
reference well within the 2e-2 relative-error gate.)

Device mapping: each of the B*H = 16384 (batch, hidden) pairs is an
independent length-T recurrence. We transpose host-side to (B*H, T)
row-major fp16, shard 2048 rows to each of the 8 cores, and on each
core run the recurrence with rows on SBUF partitions and time on the
free dimension using the VectorE `tensor_tensor_scan` instruction
(state = c * state + v along the free dim; the scan's internal state is
fp32 regardless of operand dtype, per-partition initial). ScalarE does
the two exps; SyncE issues input DMA, GpSimdE issues output DMA. All
DMA is contiguous. The kernel is DMA-bound: fp16 I/O moves ~48 MB/core
(16 MB each of log_coeffs, log_values, out) vs ~96 MB for fp32,
halving the HBM traffic. fp16 quantization of the log-space inputs and
the fp16 output downcast contribute ~5e-3 worst-case relative error,
well inside the 2e-2 gate.

Raw-bass pipeline (no Tile): NBUF-deep ring buffers per stream with
explicit semaphores, exp computed in place (c overwrites the lc tile,
v overwrites the lv tile).  Per chunk i:
    SP:   [wait slot free]  load lc_i -> cbuf,  load lv_i -> vbuf
    ACT:  [wait lc_i]  c_i = exp(lc_i)   [wait lv_i]  v_i = exp(lv_i)
    DVE:  [wait c_i,v_i; wait hbuf free] h_i = scan(c_i, v_i, init)
    POOL: [wait h_i] store h_i -> out
`init` is v_i[:,0:1] for a group's first chunk, else h_{i-1}[:,-1:]
(consecutive scans of one row-group run in order on the DVE).
"""

import contextlib

import numpy as np

import concourse.bass as bass
import concourse.mybir as mybir
from concourse.bass_utils import run_bass_kernel_spmd

B, T, H = 16, 4096, 1024
N_CORES = 8
ROWS = B * H // N_CORES  # 2048 rows (sequences) per core
F32 = mybir.dt.float32
F16 = mybir.dt.float16


def build_nc_inplace(rows: int = ROWS, t: int = T, tc: int = 2048,
                     repeat: int = 1, nbuf: int = 8,
                     iodt=F16) -> bass.Bass:
    """Per-core SPMD program; exp computed in place (c overwrites the
    lc tile, v overwrites the lv tile): 3 SBUF streams.

    Inputs:  lc (rows, t)     log_coeffs, time-major rows, dtype iodt
             lv (rows, t+1)   log_values, time-major rows, dtype iodt
    Output:  out (rows, t)    h_1..h_t per row, dtype iodt

    `repeat` re-emits the program body (for wall-clock timing); the
    result is idempotent.
    """
    assert rows % 128 == 0 and t % tc == 0 and nbuf >= 2
    nc = bass.Bass()
    lc = nc.declare_dram_parameter("lc", [rows, t], iodt, isOutput=False)
    lv = nc.declare_dram_parameter("lv", [rows, t + 1], iodt, isOutput=False)
    out = nc.declare_dram_parameter("out", [rows, t], iodt, isOutput=True)

    n_groups = rows // 128
    n_chunks = t // tc
    n_iters = repeat * n_groups * n_chunks
    exp = mybir.ActivationFunctionType.Exp
    sched = [(g, k) for _ in range(repeat) for g in range(n_groups)
             for k in range(n_chunks)]

    with contextlib.ExitStack() as ctx:
        def sb(name, width):
            return [ctx.enter_context(
                nc.sbuf_tensor(f"{name}{j}", [128, width], iodt))
                for j in range(nbuf)]

        cbuf = sb("cbuf", tc)       # holds log_coeffs, then exp'd in place
        vbuf = sb("vbuf", tc + 1)   # holds log_values, then exp'd in place
        hbuf = sb("hbuf", tc)
        # DMA completions are NOT ordered across queues, so a single
        # counting semaphore per stream is ambiguous. One semaphore per
        # ring slot -> at most one outstanding DMA per semaphore -> the
        # count is exact.
        lc_sem = [ctx.enter_context(nc.semaphore(f"lc_sem{j}")) for j in range(nbuf)]
        lv_sem = [ctx.enter_context(nc.semaphore(f"lv_sem{j}")) for j in range(nbuf)]
        out_sem = [ctx.enter_context(nc.semaphore(f"out_sem{j}")) for j in range(nbuf)]
        act_sem = ctx.enter_context(nc.semaphore("act_sem"))
        scan_sem = ctx.enter_context(nc.semaphore("scan_sem"))
        block = ctx.enter_context(nc.Block())

        @block.sync
        def _(sync: bass.BassEngine):
            for i, (g, k) in enumerate(sched):
                rs, c0 = slice(g * 128, (g + 1) * 128), k * tc
                b = i % nbuf
                if i >= nbuf:
                    # cbuf[b]/vbuf[b] last read by scan i-nbuf
                    sync.wait_ge(scan_sem, i - nbuf + 1)
                sync.dma_start(out=cbuf[b][:, :], in_=lc[rs, c0:c0 + tc]).then_inc(lc_sem[b], 16)
                sync.dma_start(out=vbuf[b][:, :], in_=lv[rs, c0:c0 + tc + 1]).then_inc(lv_sem[b], 16)

        @block.scalar
        def _(scalar: bass.BassEngine):
            for i, (g, k) in enumerate(sched):
                b = i % nbuf
                scalar.wait_ge(lc_sem[b], 16 * (i // nbuf + 1))
                nc.scalar.activation(cbuf[b][:, :], cbuf[b][:, :], exp).then_inc(act_sem, 1)
                scalar.wait_ge(lv_sem[b], 16 * (i // nbuf + 1))
                nc.scalar.activation(vbuf[b][:, :], vbuf[b][:, :], exp).then_inc(act_sem, 1)

        @block.vector
        def _(vector: bass.BassEngine):
            for i, (g, k) in enumerate(sched):
                b = i % nbuf
                vector.wait_ge(act_sem, 2 * i + 2)
                if i >= nbuf:
                    # hbuf[b] last read by store i-nbuf
                    vector.wait_ge(out_sem[b], 16 * (i // nbuf))
                if k != 0 and i > 0:
                    # the NX prefetches the scan's per-partition `initial`
                    # operand at decode time; force full completion of the
                    # predecessor scan (its tail writes the very column we
                    # read) before this instruction is decoded.
                    vector.wait_ge(scan_sem, i)
                init = vbuf[b][:, 0:1] if k == 0 else hbuf[(i - 1) % nbuf][:, tc - 1:tc]
                nc.vector.tensor_tensor_scan(
                    hbuf[b][:, :], cbuf[b][:, :], vbuf[b][:, 1:tc + 1], init,
                    mybir.AluOpType.mult, mybir.AluOpType.add,
                ).then_inc(scan_sem, 1)

        @block.gpsimd
        def _(gpsimd: bass.BassEngine):
            for i, (g, k) in enumerate(sched):
                rs, c0 = slice(g * 128, (g + 1) * 128), k * tc
                b = i % nbuf
                gpsimd.wait_ge(scan_sem, i + 1)
                gpsimd.dma_start(out=out[rs, c0:c0 + tc], in_=hbuf[b][:, :]).then_inc(out_sem[b], 16)
            for j in range(nbuf):
                rounds = (n_iters - 1 - j) // nbuf + 1 if j < n_iters else 0
                if rounds:
                    gpsimd.wait_ge(out_sem[j], 16 * rounds)

    return nc


def _shard_inputs(log_coeffs: np.ndarray, log_values: np.ndarray,
                  iodt_np=np.float16):
    """(B,T,H)/(B,T+1,H) -> per-core row-major (rows, time) shards."""
    lct = np.swapaxes(log_coeffs, 1, 2).reshape(B * H, T)
    lvt = np.swapaxes(log_values, 1, 2).reshape(B * H, T + 1)
    lct = np.ascontiguousarray(lct, dtype=iodt_np)
    lvt = np.ascontiguousarray(lvt, dtype=iodt_np)
    return [
        {"lc": lct[i * ROWS:(i + 1) * ROWS], "lv": lvt[i * ROWS:(i + 1) * ROWS]}
        for i in range(N_CORES)
    ]


def default_build(repeat: int = 1) -> bass.Bass:
    return build_nc_inplace(tc=2048, nbuf=8, repeat=repeat, iodt=F16)


def kernel(log_coeffs: np.ndarray, log_values: np.ndarray) -> np.ndarray:
    in_maps = _shard_inputs(log_coeffs, log_values)
    nc = default_build()
    try:
        results = run_bass_kernel_spmd(nc, in_maps, list(range(N_CORES))).results
    except Exception:
        # the shared device pool occasionally comes up wedged from a prior
        # process (NRT_EXEC_UNIT_UNRECOVERABLE); one retry clears it
        import time as _time
        _time.sleep(15)
        results = run_bass_kernel_spmd(nc, in_maps, list(range(N_CORES))).results
    full = np.concatenate([r["out"] for r in results], axis=0)  # (B*H, T)
    out = np.swapaxes(full.reshape(B, H, T), 1, 2)  # (B, T, H) strided view
    return np.ascontiguousarray(out, dtype=np.float32)
